# revision 28
# baseline (speedup 1.0000x reference)
"""ContrastiveCenterLoss kernel for 8x Trainium2 NeuronCores (Bass/Tile).

Math (matches the reference):
    hist = bincount(y, C) ; count = hist + 1
    dist_i = ||hidden_i - centers[y_i]||^2
    s = sum_i dist_i / count[y_i]
      = sum_c D_c / (hist_c + 1),   D_c = sum_{i: y_i = c} dist_i
    loss = 0.5 * s / (s + 1e-6)

Strategy: data-parallel over the batch (8192 samples/core).  Each core:
  - converts centers f32 -> fp8(e3m4) into a DRAM table (HWDGE + DVE)
  - streams its hidden shard (f32) and gathers the per-sample center
    rows from the fp8 table with SWDGE prepare_only descriptor prep +
    trigger_dma, so the Pool engine only pays ~1.3us/chunk of desc-gen
    and the row DMAs drain asynchronously on the 16 SDMA engines
  - per 128-sample tile: DVE subtract, ACT square+accumulate -> dist
  - bins (dist, 1) per class with a hi/lo class-id split: two tiny
    one-hots [128,8]/[128,128] and one PE matmul accumulating a
    [16,128] f32 PSUM tile (rows 0:8 = D bins, rows 8:16 = hist bins)
Host combines the 8 per-core [16,128] partials: s = sum D/(hist+1),
loss = 0.5*s/(s+eps).
"""

import numpy as np

B = 65536
D = 512
C = 1000
NCORES = 8
BLOC = B // NCORES          # 8192 samples per core
P = 128                     # partitions
SLOTS = BLOC // P           # 64 sample-slots per partition
PLAN = [(i * 8, 8) for i in range(8)]  # (start_slot, n_slots)
CHUNKS = len(PLAN)
QPC = 8                     # max slots per chunk (tile sizing)
NIDX = P * QPC
HI = 8                      # class-id high part (c >> 7), 0..7
LO = 128                    # class-id low part (c & 127)
LAMBDA_C = 1.0
EPS = 1e-6

_CACHE = {}


def _build_program():
    import concourse.bacc as bacc
    import concourse.bass as bass
    import concourse.tile as tile
    from concourse import library_config, mybir

    f32 = mybir.dt.float32
    i32 = mybir.dt.int32
    f16 = mybir.dt.bfloat16
    f8 = mybir.dt.float8e3
    i16 = mybir.dt.int16
    Alu = mybir.AluOpType
    Act = mybir.ActivationFunctionType

    nc = bacc.Bacc(
        "TRN2",
        target_bir_lowering=False,
        debug=False,
        enable_asserts=False,
        num_devices=NCORES,
    )

    hidden_ap = nc.dram_tensor("hidden", [BLOC, D], f32, kind="ExternalInput").ap()
    centers_ap = nc.dram_tensor("centers", [C, D], f32, kind="ExternalInput").ap()
    ypg_ap = nc.dram_tensor("ypg", [P, SLOTS], i32, kind="ExternalInput").ap()
    yidx_ap = nc.dram_tensor("yidx", [P, 8 * SLOTS], i16, kind="ExternalInput").ap()
    # class-id iota tables come from the host: computing them with
    # gpsimd.iota would switch the Q7 library away from mlp and back,
    # costing ~30us of ucode reload DMA before the first dma_gather
    clslo_ap = nc.dram_tensor("clslo", [P, LO], f16, kind="ExternalInput").ap()
    clshi_ap = nc.dram_tensor("clshi", [P, HI], f16, kind="ExternalInput").ap()
    out_ap = nc.dram_tensor("out", [2 * HI, LO], f32, kind="ExternalOutput").ap()

    # sample (p, t) of this core's shard is shard row p*SLOTS + t
    hview = hidden_ap.rearrange("(p t) d -> p t d", p=P)

    with tile.TileContext(nc) as tc:
        with (
            tc.tile_pool(name="persist", bufs=1) as persist,
            tc.tile_pool(name="hpool", bufs=3) as hpool,
            tc.tile_pool(name="cpool", bufs=4) as cpool,
            tc.tile_pool(name="dpool", bufs=4) as dpool,
            tc.tile_pool(name="spool", bufs=4) as spool,
            tc.tile_pool(name="lpool", bufs=4) as lpool,
            tc.tile_pool(name="opool", bufs=4) as opool,
            tc.tile_pool(name="psum", bufs=1, space="PSUM") as psum,
            tc.tile_pool(name="dram", bufs=1, space="DRAM") as dram,
        ):
            nc.gpsimd.load_library(library_config.mlp)
            with tc.high_priority():
                # aux loads ride the Scalar engine's HWDGE queue so they
                # never sit behind the hidden stream on the sync queue.
                # yidx first: it gates the gather chain.
                yidx = persist.tile([P, 8 * SLOTS], i16)
                nc.scalar.dma_start(yidx[:], yidx_ap[:])
                ypg = persist.tile([P, SLOTS], i32)
                nc.scalar.dma_start(ypg[:], ypg_ap[:])
                cls_lo = persist.tile([P, LO], f16)
                nc.scalar.dma_start(cls_lo[:], clslo_ap[:])
                cls_hi = persist.tile([P, HI], f16)
                nc.scalar.dma_start(cls_hi[:], clshi_ap[:])

                # fp8 centers table in DRAM (quarters gather HBM traffic).
                # The SDMA engines serve the sync ring FIFO, so the table
                # load/store are placed IN that ring, sandwiched around h0,
                # instead of on a separate ring where the 16MiB hidden
                # stream would starve them until ~55us.
                CP = C // 8  # 125 partitions
                ctab = dram.tile([C, D], f8)
                cstage32 = persist.tile([CP, 8 * D], f32)
                nc.sync.dma_start(
                    cstage32[:], centers_ap.rearrange("(p j) d -> p (j d)", p=CP)
                )
                # cast on ACT: the in-order DVE stream would park this
                # behind data-dependent chunk work and stall the table
                cstage8 = persist.tile([CP, 8 * D], f8)
                nc.scalar.copy(cstage8[:], cstage32[:])

            # y -> (hi, lo) split, as f32 per-partition scalar sources
            ylo_i = persist.tile([P, SLOTS], i32)
            nc.vector.tensor_scalar(ylo_i[:], ypg[:], 127, None, op0=Alu.bitwise_and)
            yhi_i = persist.tile([P, SLOTS], i32)
            nc.vector.tensor_scalar(
                yhi_i[:], ypg[:], 7, None, op0=Alu.logical_shift_right
            )
            ylo = persist.tile([P, SLOTS], f16)
            nc.vector.tensor_copy(ylo[:], ylo_i[:])
            yhi = persist.tile([P, SLOTS], f16)
            nc.vector.tensor_copy(yhi[:], yhi_i[:])

            dist = persist.tile([P, SLOTS], f32)
            acc = psum.tile([2 * HI, LO], f32)

            # sync-ring order: h0 first, then the ctab store (right behind
            # the centers load + h0 in the FIFO, ready ~20us), then h1-h7.
            h32s = []
            for g, (s0, ns) in enumerate(PLAN):
                sl = slice(s0, s0 + ns)
                h32 = hpool.tile([P, QPC, D], f32)
                h32s.append(h32)
                nc.sync.dma_start(h32[:, :ns, :], hview[:, sl, :])
                if g == 0:
                    nc.sync.dma_start(
                        ctab[:].rearrange("(p j) d -> p (j d)", p=CP), cstage8[:]
                    )

            # issue every per-chunk gather up-front so the Pool engine's
            # SWDGE chain (the ~9ns/descriptor wall) starts as soon as its
            # source is ready and runs ahead of the compute pipeline
            c8s = []
            for g, (s0, ns) in enumerate(PLAN):
                c8 = cpool.tile([P, QPC, D], f8, name=f"c8_{g}", bufs=1)
                c8s.append(c8)
                nc.gpsimd.dma_gather(
                    c8[:, :ns, :],
                    ctab[:],
                    yidx[:, 8 * s0 : 8 * (s0 + ns)],
                    num_idxs=P * ns,
                    num_idxs_reg=P * ns,
                    elem_size=D,
                    single_packet=True,
                )

            for g, (s0, ns) in enumerate(PLAN):
                sl = slice(s0, s0 + ns)
                h32 = h32s[g]
                c8 = c8s[g]

                # whole-chunk subtract (one DVE op); h f32, c fp8, out bf16
                diff = dpool.tile([P, QPC, D], f16)
                nc.vector.tensor_sub(
                    diff[:, :ns, :].rearrange("p q d -> p (q d)"),
                    h32[:, :ns, :].rearrange("p q d -> p (q d)"),
                    c8[:, :ns, :].rearrange("p q d -> p (q d)"),
                )

                # per-tile square + accumulate -> dist column (ACT)
                for q in range(ns):
                    t = s0 + q
                    sq = spool.tile([P, D], f16)
                    nc.scalar.activation(
                        sq[:], diff[:, q, :], Act.Square,
                        accum_out=dist[:, t : t + 1],
                    )

                dist16 = lpool.tile([P, QPC], f16, name=f"dist16_{g}")
                nc.scalar.copy(dist16[:, :ns], dist[:, sl])

                # batched one-hots for the whole chunk
                lhsT = lpool.tile([P, QPC, 2 * HI], f16)
                nc.vector.tensor_tensor(
                    lhsT[:, :ns, HI:],
                    cls_hi[:].unsqueeze(1).to_broadcast([P, ns, HI]),
                    yhi[:, sl].unsqueeze(2).to_broadcast([P, ns, HI]),
                    op=Alu.is_equal,
                )
                nc.vector.tensor_tensor(
                    lhsT[:, :ns, :HI],
                    lhsT[:, :ns, HI:],
                    dist16[:, :ns].unsqueeze(2).to_broadcast([P, ns, HI]),
                    op=Alu.mult,
                )
                ohlo = opool.tile([P, QPC, LO], f16)
                nc.vector.tensor_tensor(
                    ohlo[:, :ns, :],
                    cls_lo[:].unsqueeze(1).to_broadcast([P, ns, LO]),
                    ylo[:, sl].unsqueeze(2).to_broadcast([P, ns, LO]),
                    op=Alu.is_equal,
                )

                for q in range(ns):
                    t = s0 + q
                    nc.tensor.matmul(
                        out=acc[:],
                        lhsT=lhsT[:, q, :],
                        rhs=ohlo[:, q, :],
                        start=(t == 0),
                        stop=(t == SLOTS - 1),
                    )

            res = persist.tile([2 * HI, LO], f32)
            nc.vector.tensor_copy(res[:], acc[:])
            nc.sync.dma_start(out_ap[:], res[:])

    nc.compile()
    return nc


def _prep_core_inputs(y_shard, hidden_shard, centers):
    """Host-side layout marshaling for one core's shard."""
    ypg = y_shard.astype(np.int32).reshape(P, SLOTS)  # sample (p,t) = row p*SLOTS+t

    # dma_gather idx list for chunk g, position j = q*128 + p -> sample
    # (p, g*QPC+q); wrapped: idx j lives at [j % 16, j // 16], replicated
    # over the 8 groups of 16 partitions.
    cols = []
    for s0, ns in PLAN:
        flat = ypg[:, s0 : s0 + ns].T.reshape(P * ns)
        wrapped = flat.reshape(P * ns // 16, 16).T
        cols.append(np.tile(wrapped, (P // 16, 1)))
    yidx = np.concatenate(cols, axis=1).astype(np.int16)

    import ml_dtypes

    bf16 = ml_dtypes.bfloat16
    clslo = np.broadcast_to(np.arange(LO, dtype=np.float32), (P, LO)).astype(bf16)
    clshi = np.broadcast_to(np.arange(HI, dtype=np.float32), (P, HI)).astype(bf16)

    return {
        "hidden": np.ascontiguousarray(hidden_shard, dtype=np.float32),
        "centers": np.ascontiguousarray(centers, dtype=np.float32),
        "ypg": np.ascontiguousarray(ypg),
        "yidx": np.ascontiguousarray(yidx),
        "clslo": np.ascontiguousarray(clslo),
        "clshi": np.ascontiguousarray(clshi),
    }


def combine_partials(outs):
    """outs: list of [16, 128] f32 per core -> scalar loss (f32)."""
    total = np.zeros((2 * HI, LO), dtype=np.float64)
    for o in outs:
        total += o.astype(np.float64)
    Dbins = total[:HI].reshape(HI * LO)[:C]
    hist = total[HI:].reshape(HI * LO)[:C]
    s = float(np.sum(Dbins / (hist + 1.0)))
    try:
        # match the reference's XLA f32 division rounding exactly
        import jax.numpy as jnp

        s32 = jnp.float32(s)
        loss = jnp.float32(LAMBDA_C / 2.0) * s32 / (s32 + jnp.float32(EPS))
        return np.asarray(loss, dtype=np.float32)
    except Exception:
        return np.float32((LAMBDA_C / 2.0) * s / (s + EPS))


def kernel(y, hidden, centers):
    from concourse.bass_utils import run_bass_kernel_spmd

    y = np.asarray(y).astype(np.int32)
    hidden = np.asarray(hidden, dtype=np.float32)
    centers = np.asarray(centers, dtype=np.float32)

    if "nc" not in _CACHE:
        _CACHE["nc"] = _build_program()
    nc = _CACHE["nc"]

    in_maps = [
        _prep_core_inputs(
            y[k * BLOC : (k + 1) * BLOC],
            hidden[k * BLOC : (k + 1) * BLOC],
            centers,
        )
        for k in range(NCORES)
    ]

    res = run_bass_kernel_spmd(nc, in_maps, core_ids=list(range(NCORES)))
    outs = [r["out"] for r in res.results]
    return combine_partials(outs)
